# revision 1
# baseline (speedup 1.0000x reference)
"""MixHop layer (hop0 + A@h1 + A^2@h2) on 8 trn2 NeuronCores.

Strategy: 1D node partition (rows) across 8 cores, with a host-side global
row permutation that load-balances edges across cores and 128-row windows
(output is inverse-permuted on the host). Dense hop matmuls on TensorE.
SpMM = dma_gather of neighbor features (bf16, 512B rows, 4 SWDGE queues) +
one-hot scatter matmuls on TensorE accumulating into per-window PSUM tiles.
The one-hot-scaled stationary tile P_T[e, r] = val_e * (r == row_off_e) is
precomputed on the host (bf16) and streamed in with large DMAs. Cross-core
halo handled by two AllGathers (hcat=[h1|h2] bf16, g bf16).

v2: dense phase computes h1|h2 first so AG1 starts ~25us earlier; the hop-0
matmuls + out0 writes run under AG1 on the otherwise idle TensorE. Each
(group, parity) gather is split into two SWDGE calls on different queues
(all four Q7 desc-gen pairs active per group). An optional prepare_only
trigger pipeline (KM_PREP=1) preps gather descriptors ahead and fires them
with aged count=None triggers; it validates on HW but does not yet beat the
legacy path (trigger retire-waits and num_idxs_reg MOVE register hazards
serialize the Pool stream), so it is off by default.
"""
import heapq
import os
import sys

for p in ("/opt/trn_rl_repo", "/root/.axon_site/_ro/trn_rl_repo"):
    if os.path.isdir(p) and p not in sys.path:
        sys.path.append(p)

import numpy as np
import ml_dtypes

N = 50000
E = 600000
C = 128
CORES = 8
NW = 50                   # windows per core
RPC = NW * 128            # 6400 rows per core (padded)
NP = RPC * CORES          # 51200
_SIZES = [1, 1] + [2] * 24  # ramped supergroups (sum = 50)
GROUPS = []
_w = 0
for _s in _SIZES:
    GROUPS.append((_w, min(NW, _w + _s)))
    _w += _s
    if _w >= NW:
        break
SG = None
NQ = 4                    # SWDGE queues
GBUFS = 8                 # gather tile buffers per parity

TRACE = False
STAGES = int(os.environ.get("KM_STAGES", "5"))
PT_DVE = os.environ.get("KM_PT", "dma") == "dve"
AG_CHUNKS = int(os.environ.get("KM_AGCH", "1"))   # AllGather chunks per parity
PREP = int(os.environ.get("KM_PREP", "0"))        # prepare_only gather pipeline
SP = bool(int(os.environ.get("KM_SP", "0")))      # single_packet gathers
EVP_BUFS = int(os.environ.get("KM_EVP", "4"))
LAG = int(os.environ.get("KM_LAG", "4"))
_CACHE = {}


def _balance_perm(edge_row, edge_col):
    """Assign nodes to (core, window) slots balancing per-(slot, parity)
    edge counts. Returns perm[new_pos] = old_row ... actually returns
    relabel[old_row] = new_row, where new_row = core*RPC + window*128 + k.
    """
    # per-node degree by destination (row) and parity of... we balance the
    # ROW side: window load = sum over rows of deg(row) split by col parity.
    # Parity of col after relabel is unknown until relabel is fixed -> use
    # total degree for balancing (parities stay ~50/50 per window).
    deg = np.bincount(edge_row, minlength=N).astype(np.int64)
    order = np.argsort(-deg, kind="stable")  # high degree first
    nslots = CORES * NW
    # greedy: put next node into least-loaded (core,window) with space
    loads = [(0, s) for s in range(nslots)]
    heapq.heapify(loads)
    space = np.full(nslots, 128, np.int64)
    new_of_old = np.empty(NP, np.int64)
    fill_ptr = np.zeros(nslots, np.int64)
    for r in order:
        while True:
            load, s = heapq.heappop(loads)
            if space[s] > 0:
                break
        k = 128 - space[s]
        space[s] -= 1
        new_of_old[r] = s * 128 + k
        if space[s] > 0:
            heapq.heappush(loads, (load + deg[r], s))
    # pad nodes fill remaining slots
    rem = []
    for s in range(nslots):
        for k in range(128 - space[s], 128):
            rem.append(s * 128 + k)
    new_of_old[N:] = rem
    return new_of_old


def _build_plan(edge_row, edge_col, edge_val):
    relabel = _balance_perm(edge_row, edge_col)
    er = relabel[edge_row]
    ec = relabel[edge_col]

    core = er // RPC
    w = (er % RPC) // 128
    off = (er % 128).astype(np.int64)
    par = ((ec % 128) // 64).astype(np.int64)
    gidx = ((ec // 128) * 64 + (ec % 64)).astype(np.int16)

    gid = (core * NW + w) * 2 + par
    ngroups = CORES * NW * 2
    counts = np.bincount(gid, minlength=ngroups).reshape(CORES, NW, 2)
    Bw = np.maximum(1, ((counts.max(axis=0) + 127) // 128))  # [NW, 2]

    cstart = np.zeros((NW, 2), np.int64)
    calls = []
    cpos = 0
    for (w0, w1) in GROUPS:
        for p in (0, 1):
            ws = list(range(w0, w1))
            nch = int(Bw[w0:w1, p].sum())
            for wi in ws:
                cstart[wi, p] = cpos
                cpos += int(Bw[wi, p])
            calls.append(dict(par=p, ws=ws, cstart=cpos - nch, nch=nch))
    T = cpos

    order = np.argsort(gid, kind="stable")
    gs = np.zeros(ngroups + 1, np.int64)
    np.cumsum(counts.reshape(-1), out=gs[1:])
    rank = np.arange(E, dtype=np.int64) - gs[gid[order]]
    pos = cstart[w[order], par[order]] * 128 + rank
    flat = core[order] * (T * 128) + pos

    idx_p = np.zeros(CORES * T * 128, np.int16)
    idx_p[flat] = gidx[order]
    idx_p = idx_p.reshape(CORES, T, 128)

    pt = np.zeros((CORES * T * 128, 128), ml_dtypes.bfloat16)
    pt[flat, off[order]] = edge_val[order].astype(ml_dtypes.bfloat16)
    pt = pt.reshape(CORES, T, 128, 128).transpose(0, 2, 1, 3)
    pt = np.ascontiguousarray(pt.reshape(CORES, 128, T * 128))

    seg = idx_p.reshape(CORES, T * 128 // 16, 16)
    wrapped16 = seg.transpose(0, 2, 1)
    gidx_w = np.ascontiguousarray(np.tile(wrapped16, (1, 8, 1)))

    off_p = np.zeros(CORES * T * 128, np.float32)
    val_p = np.zeros(CORES * T * 128, np.float32)
    off_p[flat] = off[order].astype(np.float32)
    val_p[flat] = edge_val[order]
    off_tab = np.ascontiguousarray(
        off_p.reshape(CORES, T, 128).transpose(0, 2, 1))
    val_tab = np.ascontiguousarray(
        val_p.reshape(CORES, T, 128).transpose(0, 2, 1))
    return dict(Bw=Bw, cstart=cstart, calls=calls, T=T,
                pt=pt, gidx_w=gidx_w, relabel=relabel,
                off_tab=off_tab, val_tab=val_tab)


def _build_program(plan):
    import concourse.bass as bass
    import concourse.bacc as bacc
    import concourse.mybir as mybir
    import concourse.tile as tile

    dt = mybir.dt
    Bw, cstart, calls, T = plan["Bw"], plan["cstart"], plan["calls"], plan["T"]

    nc = bacc.Bacc("TRN2", target_bir_lowering=False, debug=False,
                   num_devices=CORES, num_swdge_queues=NQ)

    xT_d = nc.dram_tensor("xT", [128, RPC], dt.bfloat16, kind="ExternalInput")
    wb_d = nc.dram_tensor("wb", [128, 768], dt.bfloat16, kind="ExternalInput")
    pt_d = None
    if not PT_DVE:
        pt_d = nc.dram_tensor("ptt", [128, T * 128], dt.bfloat16, kind="ExternalInput")
    gix_d = nc.dram_tensor("gixt", [128, T * 8], dt.int16, kind="ExternalInput")
    if PT_DVE:
        off_d = nc.dram_tensor("offt", [128, T], dt.float32, kind="ExternalInput")
        val_d = nc.dram_tensor("valt", [128, T], dt.float32, kind="ExternalInput")
        iota_d = nc.dram_tensor("iota", [128, 128], dt.float32, kind="ExternalInput")
    out0_d = nc.dram_tensor("out0", [128, NW, 128], dt.float32, kind="ExternalOutput")
    out1_d = nc.dram_tensor("out1", [128, NW, 128], dt.float32, kind="ExternalOutput")
    out2_d = nc.dram_tensor("out2", [128, NW, 128], dt.float32, kind="ExternalOutput")

    qn = [0]
    q_dirty = [[] for _ in range(NQ)]
    NSEM = 32
    dma_sems = None
    if PREP:
        dma_sems = [nc.alloc_semaphore(f"gdma{s}") for s in range(NSEM)]

    with tile.TileContext(nc) as tc:
        with (
            tc.tile_pool(name="const", bufs=1) as constp,
            tc.tile_pool(name="gath", bufs=GBUFS) as gathp,
            tc.tile_pool(name="pt", bufs=2) as ptp,
            tc.tile_pool(name="ev", bufs=EVP_BUFS) as evp,
            tc.tile_pool(name="psum", bufs=4, space="PSUM") as psp,
            tc.tile_pool(name="psd", bufs=4, space="PSUM") as psdp,
            tc.tile_pool(name="dram", bufs=1, space="DRAM") as dramp,
        ):
            xT = constp.tile([128, RPC], dt.bfloat16)
            nc.sync.dma_start(xT[:], xT_d[:])
            wb = constp.tile([128, 768], dt.bfloat16)
            nc.sync.dma_start(wb[:], wb_d[:])
            gixt = constp.tile([128, T * 8], dt.int16)
            nc.sync.dma_start(gixt[:], gix_d[:])
            ones = constp.tile([1, 128], dt.bfloat16)
            nc.vector.memset(ones[:], 1.0)
            if PREP:
                for s in range(NSEM):
                    nc.gpsimd.sem_clear(dma_sems[s])
            if PT_DVE:
                offt = constp.tile([128, T], dt.float32)
                nc.sync.dma_start(offt[:], off_d[:])
                valt = constp.tile([128, T], dt.float32)
                nc.sync.dma_start(valt[:], val_d[:])
                iota = constp.tile([128, 128], dt.float32)
                nc.sync.dma_start(iota[:], iota_d[:])

            hcat_sh = [dramp.tile([RPC // 2, 256], dt.bfloat16, name=f"hsh{p}")
                       for p in (0, 1)]
            hcat_fl = [dramp.tile([NP // 2, 256], dt.bfloat16,
                                  addr_space="Shared", name=f"hfl{p}")
                       for p in (0, 1)]
            g_sh = [dramp.tile([RPC // 2, 128], dt.bfloat16, name=f"gsh{p}")
                    for p in (0, 1)]
            g_fl = [dramp.tile([NP // 2, 128], dt.bfloat16,
                               addr_space="Shared", name=f"gfl{p}")
                    for p in (0, 1)]

            # ---- dense h1|h2 phase, batched per DG windows; AG1 chunked ----
            DG = 5
            ag_bounds = [round(i * NW / AG_CHUNKS) for i in range(AG_CHUNKS + 1)]
            next_ag = 0
            for w0 in range(0, NW, DG):
                nwg = min(DG, NW - w0)
                h1b = evp.tile([128, nwg, 128], dt.bfloat16, tag="h1")
                h2b = evp.tile([128, nwg, 128], dt.bfloat16, tag="h2")
                for wl in range(nwg):
                    w = w0 + wl
                    ph = psdp.tile([128, 256], dt.float32, tag="ph", bufs=2)
                    nc.tensor.matmul(ph[:], ones[:], wb[0:1, 512:768],
                                     start=True, stop=False)
                    for j in (1, 2):
                        nc.tensor.matmul(ph[:, (j - 1) * 128:j * 128],
                                         xT[:, w * 128:(w + 1) * 128],
                                         wb[:, j * 128:(j + 1) * 128],
                                         start=False, stop=(j == 2))
                    nc.vector.tensor_copy(h1b[:, wl, :], ph[:, 0:128])
                    nc.vector.tensor_copy(h2b[:, wl, :], ph[:, 128:256])
                # node (w, p) -> parity p%2, local row w*64 + p//2
                # hcat row layout per node: [h1(128) | h2(128)]
                for par in (0, 1):
                    hv = hcat_sh[par][w0 * 64:(w0 + nwg) * 64, :].rearrange(
                        "(g a) (j c) -> a g j c", a=64, j=2)
                    nc.sync.dma_start(hv[:, :, 0, :],
                                      h1b[par * 64:(par + 1) * 64, :, :])
                    nc.sync.dma_start(hv[:, :, 1, :],
                                      h2b[par * 64:(par + 1) * 64, :, :])
                if STAGES >= 2:
                    while (next_ag < AG_CHUNKS
                           and w0 + nwg >= ag_bounds[next_ag + 1]):
                        a64 = ag_bounds[next_ag] * 64
                        b64 = ag_bounds[next_ag + 1] * 64
                        for par in (0, 1):
                            ov = hcat_fl[par][:].rearrange(
                                "(c r) f -> c r f", c=CORES)[:, a64:b64, :]
                            nc.gpsimd.collective_compute(
                                "AllGather", mybir.AluOpType.bypass,
                                replica_groups=[list(range(CORES))],
                                ins=[hcat_sh[par][a64:b64, :].opt()],
                                outs=[ov.opt()])
                        next_ag += 1

            # ---- h0 phase (Tensor work overlapping AG1 on the CC cores) ----
            for w0 in range(0, NW, DG):
                nwg = min(DG, NW - w0)
                h0b = evp.tile([128, nwg, 128], dt.float32, tag="h0")
                for wl in range(nwg):
                    w = w0 + wl
                    ph0 = psdp.tile([128, 128], dt.float32, tag="ph0",
                                    bufs=2)
                    nc.tensor.matmul(ph0[:], ones[:], wb[0:1, 384:512],
                                     start=True, stop=False)
                    nc.tensor.matmul(ph0[:], xT[:, w * 128:(w + 1) * 128],
                                     wb[:, 0:128], start=False, stop=True)
                    nc.vector.tensor_copy(h0b[:, wl, :], ph0[:])
                nc.sync.dma_start(out0_d[:, w0:w0 + nwg, :], h0b[:])

            def spmm_pass(src_fl, elem, out_cols, oud, evict_g, ag_after=None):
                pend = {}

                def issue_gather(gi, p):
                    call = calls[gi * 2 + p]
                    nch = call["nch"]
                    cs = call["cstart"]
                    gt = gathp.tile([128, nch, elem], dt.bfloat16,
                                    tag=f"g{p}", name=f"gt{p}",
                                    bufs=LAG + 3 if p == 0 else LAG + 2)
                    h = nch // 2
                    for (a, b) in ((0, h), (h, nch)):
                        if b <= a:
                            continue
                        nc.gpsimd.dma_gather(
                            gt[:, a:b, :], src_fl[p][:, :],
                            gixt[:, (cs + a) * 8:(cs + b) * 8],
                            num_idxs=(b - a) * 128,
                            num_idxs_reg=(b - a) * 128,
                            elem_size=elem, elem_step=elem,
                            single_packet=SP, queue_num=qn[0] % NQ)
                        qn[0] += 1
                    return (gt, cs)

                def prep_call(gi, p):
                    call = calls[gi * 2 + p]
                    nch = call["nch"]
                    cs = call["cstart"]
                    gt = gathp.tile([128, nch, elem], dt.bfloat16,
                                    tag=f"g{p}", name=f"gt{p}",
                                    bufs=LAG + 3 if p == 0 else LAG + 2)
                    g = qn[0]
                    q = g % NQ
                    qn[0] += 1
                    nc.gpsimd.dma_gather(
                        gt[:], src_fl[p][:, :],
                        gixt[:, cs * 8:(cs + nch) * 8],
                        num_idxs=nch * 128, num_idxs_reg=nch * 128,
                        elem_size=elem, elem_step=elem,
                        single_packet=SP, queue_num=q,
                        prepare_only=True,
                        sem=dma_sems[g % NSEM])
                    q_dirty[q].append(gt[:])
                    pend.setdefault(gi, {})[p] = (gt, cs, g % NSEM,
                                                  g // NSEM + 1)

                def trigger_dirty(queues=None):
                    for q in (range(NQ) if queues is None else queues):
                        if q_dirty[q]:
                            nc.gpsimd.trigger_dma(count=None, queue_num=q)
                            q_dirty[q] = []

                def issue_ptt(gi):
                    c0 = calls[gi * 2]["cstart"]
                    c1 = calls[gi * 2 + 1]["cstart"] + calls[gi * 2 + 1]["nch"]
                    ptt = ptp.tile([128, (c1 - c0) * 128], dt.bfloat16,
                                   tag="ptt", name="ptt", bufs=3)
                    if PT_DVE:
                        for cg in range(c0, c1):
                            nc.vector.tensor_scalar(
                                ptt[:, (cg - c0) * 128:(cg - c0 + 1) * 128],
                                iota[:], offt[:, cg:cg + 1],
                                valt[:, cg:cg + 1],
                                mybir.AluOpType.is_equal,
                                mybir.AluOpType.mult)
                    else:
                        nc.scalar.dma_start(ptt[:], pt_d[:, c0 * 128:c1 * 128])
                    pend.setdefault(gi, {})["ptt"] = ptt

                def _process_group(gi):
                    w0, w1 = GROUPS[gi]
                    nwg = w1 - w0
                    c0 = calls[gi * 2]["cstart"]
                    c1 = calls[gi * 2 + 1]["cstart"] + calls[gi * 2 + 1]["nch"]
                    gts = pend.pop(gi)
                    ptt = gts.pop("ptt")
                    if PREP:
                        for p in (0, 1):
                            _, _, si, rep = gts[p]
                            nc.tensor.wait_ge(dma_sems[si], 16 * rep)
                    ycb = evp.tile([128, nwg, 128], dt.float32, tag="yc",
                                   name="ycb")
                    gcb = None
                    if evict_g:
                        gcb = evp.tile([128, nwg, 128], dt.bfloat16, tag="gc",
                                       name="gcb")
                    for w in range(w0, w1):
                        nchw = int(Bw[w, 0] + Bw[w, 1])
                        ps = psp.tile([128, out_cols], dt.float32, tag="ps")
                        k = 0
                        for p in (0, 1):
                            gt, cs = gts[p][0], gts[p][1]
                            for bch in range(int(Bw[w, p])):
                                cg = int(cstart[w, p]) + bch
                                lp = cg - cs
                                nc.tensor.matmul(
                                    ps[:],
                                    ptt[:, (cg - c0) * 128:(cg - c0 + 1) * 128],
                                    gt[:, lp, :],
                                    start=(k == 0), stop=(k == nchw - 1))
                                k += 1
                        nc.vector.tensor_copy(ycb[:, w - w0, :], ps[:, 0:128])
                        if evict_g:
                            nc.vector.tensor_copy(gcb[:, w - w0, :],
                                                  ps[:, 128:256])
                    nc.sync.dma_start(oud[:, w0:w1, :], ycb[:])
                    if evict_g:
                        for par in (0, 1):
                            gv = g_sh[par][w0 * 64:w1 * 64, :].rearrange(
                                "(g a) c -> a g c", a=64)
                            nc.scalar.dma_start(
                                gv[:], gcb[par * 64:(par + 1) * 64, :, :])

                nG = len(GROUPS)
                if PREP:
                    PD = max(LAG, 2)
                    for gj in range(min(PD, nG)):
                        prep_call(gj, 0)
                        prep_call(gj, 1)
                    issue_ptt(0)
                    for gi in range(nG):
                        # fire only aged pending (the queues about to be
                        # re-prepped) so the trigger's wait-for-prep-retire
                        # never drains the in-flight desc-gen pipeline
                        if gi + PD < nG:
                            trigger_dirty([qn[0] % NQ, (qn[0] + 1) % NQ])
                        else:
                            trigger_dirty()
                        if gi + 1 < nG:
                            issue_ptt(gi + 1)
                        if gi + PD < nG:
                            prep_call(gi + PD, 0)
                            prep_call(gi + PD, 1)
                        _process_group(gi)
                        if ag_after and gi in ag_after:
                            ag_after[gi]()
                else:
                    for gi in range(nG + LAG):
                        if gi < nG:
                            pend[gi] = {0: issue_gather(gi, 0)}
                        ok = gi - (LAG - 2)
                        if 0 <= ok < nG:
                            pend[ok] = pend.get(ok, {})
                            pend[ok][1] = issue_gather(ok, 1)
                            issue_ptt(ok)
                        pk = gi - LAG
                        if 0 <= pk < nG:
                            _process_group(pk)
                            if ag_after and pk in ag_after:
                                ag_after[pk]()

            ag2_hooks = {}
            if STAGES >= 4:
                bounds = [round(i * NW / AG_CHUNKS)
                          for i in range(AG_CHUNKS + 1)]

                def mk_ag2(ci):
                    a64 = bounds[ci] * 64
                    b64 = bounds[ci + 1] * 64

                    def hook():
                        for par in (0, 1):
                            ov = g_fl[par][:].rearrange(
                                "(c r) f -> c r f", c=CORES)[:, a64:b64, :]
                            nc.gpsimd.collective_compute(
                                "AllGather", mybir.AluOpType.bypass,
                                replica_groups=[list(range(CORES))],
                                ins=[g_sh[par][a64:b64, :].opt()],
                                outs=[ov.opt()])
                    return hook

                for ci in range(AG_CHUNKS):
                    for gi, (w0, w1) in enumerate(GROUPS):
                        if w1 >= bounds[ci + 1]:
                            ag2_hooks.setdefault(gi, []).append(mk_ag2(ci))
                            break
                ag2_hooks = {gi: (lambda fns=fns: [f() for f in fns])
                             for gi, fns in ag2_hooks.items()}

            if STAGES >= 3:
                spmm_pass(hcat_fl, 256, 256, out1_d, True,
                          ag_after=ag2_hooks if STAGES >= 4 else None)

            if STAGES >= 5:
                spmm_pass(g_fl, 128, 128, out2_d, False)

    nc.compile()
    return nc


def _prepare_inputs(x, W, b, plan):
    relabel = plan["relabel"]
    xpad = np.zeros((NP, C), np.float32)
    xpad[relabel[:N]] = x
    xT = xpad.T
    Wp = np.concatenate([W[0], W[1], W[2]], axis=1)
    biasrow = np.zeros((128, 384), np.float32)
    biasrow[0] = np.concatenate([b[0], b[1], b[2]])
    wb = np.concatenate([Wp, biasrow], axis=1)

    in_maps = []
    for c in range(CORES):
        in_maps.append({
            "xT": np.ascontiguousarray(xT[:, c * RPC:(c + 1) * RPC]).astype(ml_dtypes.bfloat16),
            "wb": wb.astype(ml_dtypes.bfloat16),
            "ptt": plan["pt"][c],
            "gixt": plan["gidx_w"][c],
        })
        if PT_DVE:
            in_maps[-1]["offt"] = plan["off_tab"][c]
            in_maps[-1]["valt"] = plan["val_tab"][c]
            in_maps[-1]["iota"] = np.broadcast_to(
                np.arange(128, dtype=np.float32), (128, 128)).copy()
            del in_maps[-1]["ptt"]
    return in_maps


def kernel(x, W, b, edge_val, edge_row, edge_col):
    x = np.asarray(x, np.float32)
    W = np.asarray(W, np.float32)
    b = np.asarray(b, np.float32)
    edge_val = np.asarray(edge_val, np.float32)
    edge_row = np.asarray(edge_row, np.int32)
    edge_col = np.asarray(edge_col, np.int32)

    from concourse.bass_utils import run_bass_kernel_spmd

    key = hash((edge_row.tobytes(), edge_col.tobytes(), edge_val.tobytes()))
    if key not in _CACHE:
        plan = _build_plan(edge_row, edge_col, edge_val)
        nc = _build_program(plan)
        _CACHE[key] = (plan, nc)
    plan, nc = _CACHE[key]

    in_maps = _prepare_inputs(x, W, b, plan)
    res = run_bass_kernel_spmd(nc, in_maps, core_ids=list(range(CORES)),
                               trace=TRACE)
    kernel.last_results = res
    parts = []
    for c in range(CORES):
        r = res.results[c]
        blk = np.stack([r["out0"], r["out1"], r["out2"]], axis=-2)
        # blk [128 p, NW, 3, 128c] -> rows (w,p): transpose to [NW, p, 3*128]
        parts.append(blk.transpose(1, 0, 2, 3).reshape(RPC, 384))
    full = np.concatenate(parts, axis=0)
    return np.ascontiguousarray(full[plan["relabel"][:N]])


if __name__ == "__main__":
    rng = np.random.default_rng(0)
    x = rng.standard_normal((N, C), dtype=np.float32)
    W = rng.standard_normal((3, C, C), dtype=np.float32) / np.sqrt(C)
    b = rng.standard_normal((3, C), dtype=np.float32) * 0.01
    ev = rng.random(E, dtype=np.float32)
    er = rng.integers(0, N, E, dtype=np.int32)
    ec = rng.integers(0, N, E, dtype=np.int32)
    out = kernel(x=x, W=W, b=b, edge_val=ev, edge_row=er, edge_col=ec)
    print(out.shape, out.dtype)

